# revision 15
# baseline (speedup 1.0000x reference)
"""Trainium2 Bass kernel for batched multi-head self-attention (v1 fallback).

Measured: HW exec 160570 ns (traced), rel err 4.3e-4.
13-unit pipeline, PV with [V|1] ones-column (M=65), no col-tiling, f32 out.
"""

import numpy as np

import concourse.mybir as mybir
import concourse.tile as tile
from concourse import bacc
from concourse.bass_utils import run_bass_kernel_spmd

B, NT, D, H, HD = 8, 1024, 768, 12, 64
KC = D // 128
NPAIR = H // 2
SCALE = float(D) ** -0.5
F32 = mybir.dt.float32
FP16 = mybir.dt.float16
PW = 2 * (HD + 1)      # 130 V cols per pair: [V_h0 | 1 | V_h1 | 1]
VW = NPAIR * PW        # 780
OW = H * (HD + 1)      # 780 output rows


def _build():
    nc = bacc.Bacc("TRN2", target_bir_lowering=False, debug=False, num_devices=B)

    xT16 = nc.dram_tensor("xT16", [D, NT], FP16, kind="ExternalInput")
    wqk = nc.dram_tensor("wqk", [D, 2 * D], FP16, kind="ExternalInput")
    wv = nc.dram_tensor("wv", [D, VW], FP16, kind="ExternalInput")
    bqk = nc.dram_tensor("bqk", [128, H], F32, kind="ExternalInput")
    bv = nc.dram_tensor("bv", [128, VW], F32, kind="ExternalInput")
    ones = nc.dram_tensor("ones", [128, 256], FP16, kind="ExternalInput")
    outp = nc.dram_tensor("outp", [OW, NT], F32, kind="ExternalOutput")

    with tile.TileContext(nc) as tc:
        with (
            tc.tile_pool(name="res", bufs=1) as res,
            tc.tile_pool(name="expool", bufs=20) as expool,
            tc.tile_pool(name="pvtp", bufs=4) as pvtp,
            tc.tile_pool(name="eps", bufs=2, space="PSUM") as eps_p,
            tc.tile_pool(name="qkps", bufs=1, space="PSUM") as qk_ps,
            tc.tile_pool(name="pvps", bufs=3, space="PSUM") as pv_ps,
        ):
            xt16 = [res.tile([128, NT], FP16, tag=f"xt16_{k}", name=f"xt16_{k}")
                    for k in range(KC)]
            qkt = [res.tile([128, NT], FP16, tag=f"qkt{e}", name=f"qkt{e}")
                   for e in range(H)]
            vp = [res.tile([128, VW], FP16, tag=f"vp{t}", name=f"vp{t}")
                  for t in range(8)]
            wqk_sb = [[res.tile([128, 256], FP16, tag=f"wqk{p}_{k}", name=f"wqk{p}_{k}")
                       for k in range(KC)] for p in range(NPAIR)]
            wv_sb = [res.tile([128, VW], FP16, tag=f"wv{k}", name=f"wv{k}")
                     for k in range(KC)]
            bqk_sb = res.tile([128, H], F32, tag="bqk")
            bvv = res.tile([128, VW], F32, tag="bvv")
            ones_sb = res.tile([128, 256], FP16, tag="ones")

            nc.sync.dma_start(ones_sb[:], ones[:, :])
            nc.sync.dma_start(bqk_sb[:], bqk[:, :])
            nc.sync.dma_start(bvv[:], bv[:, :])
            for k in range(KC):
                nc.sync.dma_start(wqk_sb[0][k][:], wqk[k * 128:(k + 1) * 128, 0:256])
            for k in range(KC):
                nc.sync.dma_start(xt16[k][:, 0:512],
                                  xT16[k * 128:(k + 1) * 128, 0:512])
            for k in range(KC):
                nc.sync.dma_start(xt16[k][:, 512:1024],
                                  xT16[k * 128:(k + 1) * 128, 512:1024])

            # warm the PE HAM clock to 2.4 GHz during the input-DMA wait;
            # the pvps pool is otherwise unused until unit 1, so these
            # never block the first QK-projection matmuls
            warm_ps = pv_ps.tile([128, 512], F32, tag="pvps", name="warm")
            for w in range(48):
                nc.tensor.matmul(warm_ps[0:1, 0:256], ones_sb[:, 0:1],
                                 ones_sb[:, 0:256], start=True, stop=True)

            def qk_group(p, i, tcn):
                et = 2 * p + i
                ps = qk_ps.tile([128, 512], F32, tag="qkps", name="psqk")
                for k in range(KC):
                    nc.tensor.matmul(
                        ps[:, 0:512],
                        wqk_sb[p][k][:, i * 128:(i + 1) * 128],
                        xt16[k][:, tcn * 512:(tcn + 1) * 512],
                        start=(k == 0), stop=(k == KC - 1))
                nc.vector.tensor_scalar_add(
                    qkt[et][:, tcn * 512:(tcn + 1) * 512],
                    ps[:, 0:512], bqk_sb[:, et:et + 1])

            def v_group(g, t):
                ps = qk_ps.tile([128, 512], F32, tag="qkps", name="psv")
                cs = slice(g * 260, (g + 1) * 260)
                for k in range(KC):
                    nc.tensor.matmul(
                        ps[:, 0:260],
                        xt16[k][:, t * 128:(t + 1) * 128],
                        wv_sb[k][:, cs],
                        start=(k == 0), stop=(k == KC - 1))
                nc.vector.tensor_add(vp[t][:, cs], ps[:, 0:260], bvv[:, cs])

            qk_group(0, 1, 0)
            qk_group(0, 0, 0)

            for k in range(KC):
                nc.sync.dma_start(wv_sb[k][:], wv[k * 128:(k + 1) * 128, :])
            for p in range(1, NPAIR):
                for k in range(KC):
                    nc.sync.dma_start(wqk_sb[p][k][:],
                                      wqk[k * 128:(k + 1) * 128,
                                          p * 256:(p + 1) * 256])

            # QK filler groups split into 3-matmul half-chunks so one
            # filler never dilates a kt-slot much past the ScalarE exp
            # period; chunk b carries the bias copy-out.  Placement keeps
            # every producer chunk in program order before its first
            # consumer (energy slot-0 reads: due previous unit; slot-4
            # K-tcn1 reads: due slots 0-1 of the same unit).
            def qk_chunks(p, i, tcn):
                st = {}
                def a():
                    st["ps"] = qk_ps.tile([128, 512], F32, tag="qkps",
                                          name="psqk")
                    for k in range(3):
                        nc.tensor.matmul(
                            st["ps"][:, 0:512],
                            wqk_sb[p][k][:, i * 128:(i + 1) * 128],
                            xt16[k][:, tcn * 512:(tcn + 1) * 512],
                            start=(k == 0), stop=False)
                def b():
                    et = 2 * p + i
                    for k in range(3, KC):
                        nc.tensor.matmul(
                            st["ps"][:, 0:512],
                            wqk_sb[p][k][:, i * 128:(i + 1) * 128],
                            xt16[k][:, tcn * 512:(tcn + 1) * 512],
                            start=False, stop=(k == KC - 1))
                    nc.vector.tensor_scalar_add(
                        qkt[et][:, tcn * 512:(tcn + 1) * 512],
                        st["ps"][:, 0:512], bqk_sb[:, et:et + 1])
                return [(f, ()) for f in (a, b)]

            def VG(g, t):
                return [(v_group, (g, t))]

            Qc = qk_chunks
            fill = [[] for _ in range(13)]
            fill[0] = (Qc(0, 1, 1) + Qc(0, 0, 1)
                       + [(v_group, (0, t)) for t in range(8)])
            fill[1] = Qc(1, 1, 0) + Qc(1, 0, 0)
            fill[2] = (Qc(1, 1, 1) + Qc(1, 0, 1) + Qc(2, 0, 0)
                       + VG(1, 0) + VG(1, 1))
            fill[3] = (Qc(2, 0, 1) + Qc(2, 1, 0)
                       + VG(1, 2) + VG(1, 3) + VG(1, 4))
            fill[4] = (Qc(2, 1, 1) + Qc(3, 0, 0)
                       + VG(1, 5) + VG(1, 6) + VG(1, 7))
            fill[5] = Qc(3, 1, 0) + Qc(3, 0, 1)
            fill[6] = Qc(3, 1, 1) + Qc(4, 0, 0) + VG(2, 0) + VG(2, 1)
            fill[7] = (Qc(4, 1, 0) + Qc(4, 0, 1)
                       + VG(2, 2) + VG(2, 3) + VG(2, 4))
            fill[8] = (Qc(4, 1, 1) + Qc(5, 0, 0)
                       + VG(2, 5) + VG(2, 6) + VG(2, 7))
            fill[9] = Qc(5, 1, 0) + Qc(5, 0, 1)
            fill[10] = Qc(5, 1, 1)

            prev = None
            for u in range(13):
                fillers = list(fill[u])
                cur_ex = []
                if u < 12:
                    p, qc = u // 2, u % 2
                if prev is not None:
                    pp, pqc, pex = prev
                    pvps = [pv_ps.tile([128, 512], F32, tag="pvps",
                                       name=f"pvp{u}_{i}") for i in range(2)]
                for kt in range(8):
                    if prev is not None:
                        for i in range(2):
                            nc.tensor.matmul(
                                pvps[i][0:HD + 1, :],
                                vp[kt][:, pp * PW + i * (HD + 1):
                                        pp * PW + (i + 1) * (HD + 1)],
                                pex[kt][:, i * 512:(i + 1) * 512],
                                start=(kt == 0), stop=(kt == 7))
                    if u < 12:
                        eps = eps_p.tile([128, 1024], F32, tag="eps", name="eps")
                        for i in range(2):
                            nc.tensor.matmul(
                                eps[:, i * 512:(i + 1) * 512],
                                qkt[2 * p + 1][i * HD:(i + 1) * HD,
                                               kt * 128:(kt + 1) * 128],
                                qkt[2 * p][i * HD:(i + 1) * HD,
                                           qc * 512:(qc + 1) * 512],
                                start=True, stop=True)
                        ex = expool.tile([128, 1024], FP16, tag="ex", name="ex")
                        nc.scalar.activation(ex[:], eps[:],
                                             mybir.ActivationFunctionType.Exp,
                                             bias=0.0, scale=SCALE)
                        cur_ex.append(ex)
                    if fillers:
                        fn, args = fillers.pop(0)
                        fn(*args)
                for fn, args in fillers:
                    fn(*args)
                if prev is not None:
                    pp, pqc, _ = prev
                    for i in range(2):
                        h = 2 * pp + i
                        pvt = pvtp.tile([HD + 1, 512], F32, tag="pvt", name="pvt")
                        nc.vector.tensor_copy(pvt[:], pvps[i][0:HD + 1, :])
                        nc.sync.dma_start(
                            outp[h * (HD + 1):(h + 1) * (HD + 1),
                                 pqc * 512:(pqc + 1) * 512],
                            pvt[:])
                prev = (p, qc, cur_ex) if u < 12 else None

    nc.compile()
    return nc


_NC_CACHE = None


def _get_nc():
    global _NC_CACHE
    if _NC_CACHE is None:
        _NC_CACHE = _build()
    return _NC_CACHE


def _qk_perm():
    d3 = np.arange(HD) * 3
    qk_cols = []
    for p in range(NPAIR):
        for s in (0, 1):
            for h in (2 * p, 2 * p + 1):
                qk_cols.append(h * (HD * 3) + d3 + s)
    return np.concatenate(qk_cols)


def make_in_maps(x, w_qkv, b_qkv):
    qk_idx = _qk_perm()
    w32 = np.asarray(w_qkv, dtype=np.float32)
    b32 = np.asarray(b_qkv, dtype=np.float32)
    wqk = np.ascontiguousarray(w32[:, qk_idx], dtype=np.float16)
    bqk = np.ascontiguousarray(b32[qk_idx].reshape(H, 128).T)
    wv = np.zeros((D, VW), dtype=np.float16)
    bv1 = np.zeros(VW, dtype=np.float32)
    d3 = np.arange(HD) * 3
    for p in range(NPAIR):
        for i in (0, 1):
            h = 2 * p + i
            base = p * PW + i * (HD + 1)
            cols = h * (HD * 3) + d3 + 2
            wv[:, base:base + HD] = w32[:, cols].astype(np.float16)
            bv1[base:base + HD] = b32[cols]
            bv1[base + HD] = 1.0
    bv = np.ascontiguousarray(np.broadcast_to(bv1, (128, VW)))
    return [
        {
            "xT16": np.ascontiguousarray(np.asarray(x[b], dtype=np.float16).T),
            "wqk": wqk, "wv": wv, "bqk": bqk, "bv": bv,
            "ones": np.ones((128, 256), dtype=np.float16),
        }
        for b in range(B)
    ]


def postprocess(results):
    outs = []
    for b in range(B):
        pv = results[b]["outp"].reshape(H, HD + 1, NT)
        out = pv[:, :HD, :] / pv[:, HD:HD + 1, :]
        outs.append(out.transpose(2, 0, 1).reshape(NT, H * HD))
    return np.stack(outs).astype(np.float32)


def kernel(x, w_qkv, b_qkv):
    nc = _get_nc()
    in_maps = make_in_maps(x, w_qkv, b_qkv)
    res = run_bass_kernel_spmd(nc, in_maps, core_ids=list(range(B)))
    return postprocess(res.results)
